# revision 2
# baseline (speedup 1.0000x reference)
"""Trainium2 Bass kernel for nn_MultiHeadAttention_7954279432294.

Reference computation (per batch b, row h):
    qp = q^T Wq^T + bq       [W, C]   (1x1 conv channel mixing)
    kp = k^T Wk^T + bk       [W, C]
    vp = v^T Wv^T + bv       [W, C]
    out = (qp @ kp^T) @ vp   [W, C]   (linear attention, NO softmax)
    result = out^T + q       [C, W]   (NCHW + residual)

Key optimization: no softmax => reassociate (qp @ kp^T) @ vp = qp @ (kp^T @ vp),
where S = kp^T @ vp is only [C, C] = [64, 64]. This is an 8x FLOP reduction vs
materializing the [512, 512] attention matrix.

Sharding: data-parallel over B (8 batches over 8 cores), weights replicated,
no cross-device communication.

Per-core layout trick: H=64 rows are processed as 32 pairs (h, h+8) packed
into the 128 SBUF partitions (channels 0:64 = h, 64:128 = h+8), so every DMA
uses all 128 partitions and every DVE/ACT op runs at full width.
"""

import numpy as np

import concourse.bass as bass
import concourse.mybir as mybir
import concourse.tile as tile
from concourse.bass_utils import run_bass_kernel_spmd

B, C, H, W = 8, 64, 64, 512
HW = H * W
F32 = mybir.dt.float32

# chunking: 4 chunks of 16 h-rows; each chunk tile is [128, 8*512] = 2 MB
N_CHUNK = 4
H_PER_CHUNK = H // N_CHUNK          # 16
PAIRS_PER_CHUNK = H_PER_CHUNK // 2  # 8
CHUNK_F = PAIRS_PER_CHUNK * W       # 4096


def _add_bcast(nc, out_ap, in0_ap, bias_tile, reps, width=C):
    """out = in0 + bias, where bias is a [128, width] tile broadcast `reps`
    times along the free dim (out/in0 are [128, reps*width])."""
    out3 = out_ap.rearrange("p (r c) -> p r c", c=width)
    in03 = in0_ap.rearrange("p (r c) -> p r c", c=width)
    b2 = bias_tile[:, :]
    bias3 = bass.AP(
        tensor=b2.tensor,
        offset=b2.offset,
        ap=[b2.ap[0], [0, reps], b2.ap[1]],
    )
    nc.vector.tensor_tensor(
        out=out3, in0=in03, in1=bias3, op=mybir.AluOpType.add
    )


def build_nc(hw_workaround: bool = False, reps: int = 1) -> bass.Bass:
    """reps>1 repeats the whole computation inside the NEFF (idempotent) —
    used only for differential HW timing (launch overhead cancels)."""
    nc = bass.Bass()

    # weights are preprocessed host-side in kernel():
    #   Wq -> Wq^T [i, o] duplicated on both partition halves -> [128, C]
    #   Wkv -> block-diag [[Wk^T, 0], [0, Wv^T]] -> [128, 128]
    #   bq -> per-partition column duplicated -> [128, 1]
    #   bkv -> every partition = concat(bk, bv) -> [128, 128]
    q_d = nc.declare_dram_parameter("q", [C, HW], F32, isOutput=False)
    k_d = nc.declare_dram_parameter("k", [C, HW], F32, isOutput=False)
    v_d = nc.declare_dram_parameter("v", [C, HW], F32, isOutput=False)
    Wq_d = nc.declare_dram_parameter("Wq", [128, C], F32, isOutput=False)
    Wkv_d = nc.declare_dram_parameter("Wkv", [128, 128], F32, isOutput=False)
    bq_d = nc.declare_dram_parameter("bq", [128, 1], F32, isOutput=False)
    bkv_d = nc.declare_dram_parameter("bkv", [128, 128], F32, isOutput=False)
    out_d = nc.declare_dram_parameter("out", [C, HW], F32, isOutput=True)

    # chunk ch, g-half: DRAM region q[c, ch*8192 + g*4096 + e] maps to SBUF
    # partitions g*64+c. One [64, 4096] DMA per (tensor, chunk, half).
    def dram_half(d, ch, g):
        lo = ch * 2 * CHUNK_F + g * CHUNK_F
        return d[:, lo : lo + CHUNK_F]

    with tile.TileContext(nc) as tc:
        with (
            tc.tile_pool(name="const", bufs=1) as const,
            tc.tile_pool(name="io", bufs=2) as io,
            tc.tile_pool(name="mid", bufs=2) as mid,
            tc.tile_pool(name="ps2", bufs=2, space="PSUM") as ps2,
            tc.tile_pool(name="ps1", bufs=1, space="PSUM") as ps1,
        ):
            # ---------------- setup: plain DMAs (host did the prep) ----------------
            wTq = const.tile([128, C], F32)
            nc.sync.dma_start(out=wTq[:, :], in_=Wq_d[:, :])

            wkv = const.tile([128, 128], F32)
            nc.sync.dma_start(out=wkv[:, :], in_=Wkv_d[:, :])

            bq2 = const.tile([128, 1], F32)
            nc.sync.dma_start(out=bq2[:, :], in_=bq_d[:, :])

            bkv = const.tile([128, 128], F32)
            nc.sync.dma_start(out=bkv[:, :], in_=bkv_d[:, :])

            # ---------------- main loop ----------------
            # HW constraints (found empirically on this device):
            #   - consecutive matmuls may NOT switch tile_position rows unless
            #     row == col ("diagonal"); column switches are fine.
            #   Safe configs used here: (0, x) for any x, and (64, 64).
            for ch in [c for _ in range(reps) for c in range(N_CHUNK)]:
                q_sb = io.tile([128, CHUNK_F], F32, tag="q_sb")
                # kv{g}_sb rows 0:64 = k channels, 64:128 = v channels (h-group g)
                kv0_sb = io.tile([128, CHUNK_F], F32, tag="kv0_sb")
                kv1_sb = io.tile([128, CHUNK_F], F32, tag="kv1_sb")
                o_sb = io.tile([128, CHUNK_F], F32, tag="o_sb")
                lo, hi = slice(0, C), slice(C, 128)
                for g, kv_sb in ((0, kv0_sb), (1, kv1_sb)):
                    nc.sync.dma_start(out=kv_sb[lo, :], in_=dram_half(k_d, ch, g))
                    nc.sync.dma_start(out=kv_sb[hi, :], in_=dram_half(v_d, ch, g))
                    gp = slice(g * C, (g + 1) * C)
                    nc.sync.dma_start(out=q_sb[gp, :], in_=dram_half(q_d, ch, g))

                for hp in range(PAIRS_PER_CHUNK):
                    hs = slice(hp * W, (hp + 1) * W)

                    # --- fused kp+vp projection into [w, (kp|vp)] layout ---
                    # one matmul per (g, j): lhsT = [k;v] slice [128, 64],
                    # rhs = block-diag Wkv [128, 128] ->
                    # pv_ps[64g + wl, j*128 + (c | 64+c)] = (kp_g | vp_g)
                    pv_ps = ps2.tile([128, 1024], F32, tag="pv_ps")
                    for g, kv_sb in ((0, kv0_sb), (1, kv1_sb)):
                        gp = slice(g * C, (g + 1) * C)
                        for j in range(8):
                            ws = slice(hp * W + j * C, hp * W + (j + 1) * C)
                            nc.tensor.matmul(
                                pv_ps[gp, j * 128 : (j + 1) * 128],
                                kv_sb[:, ws], wkv[:, :],
                                start=True, stop=True,
                            )
                    pv_sb = mid.tile([128, 1024], F32, tag="pv_sb")
                    _add_bcast(nc, pv_sb[:, :], pv_ps[:, :], bkv, 8, width=128)

                    # --- S = kp^T @ vp  [c, c'] per h (8 accumulating K=64 chunks) ---
                    S_ps = ps1.tile([128, C], F32, tag="S_ps")
                    for g in range(2):
                        gp = slice(g * C, (g + 1) * C)
                        for j in range(8):
                            nc.tensor.matmul(
                                S_ps[gp, :],
                                pv_sb[gp, j * 128 : j * 128 + C],
                                pv_sb[gp, j * 128 + C : (j + 1) * 128],
                                start=(j == 0), stop=(j == 7),
                            )
                    S_sb = mid.tile([128, C], F32, tag="S_sb")
                    nc.scalar.copy(S_sb[:, :], S_ps[:, :])

                    # --- qp^T projection [c, w] ---
                    qp_ps = ps1.tile([128, 512], F32, tag="qp_ps")
                    for g in range(2):
                        gp = slice(g * C, (g + 1) * C)
                        nc.tensor.matmul(
                            qp_ps[gp, :], wTq[gp, :], q_sb[gp, hs],
                            start=True, stop=True,
                        )
                    qp_sb = mid.tile([128, 512], F32, tag="qp_sb")
                    nc.scalar.add(qp_sb[:, :], qp_ps[:, :], add=bq2[:, :])

                    # --- out^T[c', w] = sum_c S[c,c'] qp^T[c,w] ---
                    out_ps = ps2.tile([128, 512], F32, tag="out_ps")
                    for g in range(2):
                        gp = slice(g * C, (g + 1) * C)
                        nc.tensor.matmul(
                            out_ps[gp, :], S_sb[gp, :], qp_sb[gp, :],
                            start=True, stop=True,
                        )

                    # --- residual add, write into the output chunk tile ---
                    nc.vector.tensor_tensor(
                        out=o_sb[:, hs], in0=out_ps[:, :], in1=q_sb[:, hs],
                        op=mybir.AluOpType.add,
                    )

                for g in range(2):
                    gp = slice(g * C, (g + 1) * C)
                    nc.sync.dma_start(out=dram_half(out_d, ch, g), in_=o_sb[gp, :])

    if hw_workaround:
        _absorb_matmul_waits(nc)
    nc.finalize()
    return nc


def _absorb_matmul_waits(nc):
    """This walrus build rejects any engine instruction carrying more than one
    sync wait. Split an instruction's n waits into n same-engine NoOps (one
    wait each) inserted right before it: engines execute their stream in FIFO
    order, so the instruction stays correctly gated."""
    ctr = 0
    for bb in nc.m.functions[0].blocks:
        insts = bb.instructions
        i = 0
        while i < len(insts):
            inst = insts[i]
            si = inst.sync_info
            if si is not None and si.on_wait and len(si.on_wait) > 1:
                for w in si.on_wait:
                    nop = mybir.InstNoOp(
                        name=f"I-mmwait-{ctr}", engine=inst.engine, ins=[], outs=[]
                    )
                    ctr += 1
                    nop.sync_info = mybir.SyncInfo(on_wait=[w], on_update=[])
                    insts.insert(i, nop)
                    i += 1
                inst.sync_info = mybir.SyncInfo(
                    on_wait=[], on_update=list(si.on_update)
                )
            i += 1


_NC_CACHE = None
_RUN_KWARGS = {}   # test harness can set e.g. {"trace": True}
LAST_RESULT = None  # BassKernelResults of the last kernel() call


def _get_nc():
    global _NC_CACHE
    if _NC_CACHE is None:
        # the 1-wait workaround is needed for the HW compile path only;
        # CoreSim/TimelineSim consume a clean build_nc() module.
        _NC_CACHE = build_nc(hw_workaround=True)
    return _NC_CACHE


def prep_params(Wq, bq, Wk, bk, Wv, bv):
    Wq = np.asarray(Wq, dtype=np.float32)
    Wk = np.asarray(Wk, dtype=np.float32)
    Wv = np.asarray(Wv, dtype=np.float32)
    bq = np.asarray(bq, dtype=np.float32).reshape(C)
    bk = np.asarray(bk, dtype=np.float32).reshape(C)
    bv = np.asarray(bv, dtype=np.float32).reshape(C)

    # Wq^T duplicated on both halves -> [128, C]
    Wq_p = np.ascontiguousarray(np.concatenate([Wq.T, Wq.T], axis=0))
    # block-diag [[Wk^T, 0], [0, Wv^T]] -> [128, 128]
    Wkv = np.zeros((128, 128), dtype=np.float32)
    Wkv[0:C, 0:C] = Wk.T
    Wkv[C:128, C:128] = Wv.T
    # bq column duplicated -> [128, 1]
    bq_p = np.ascontiguousarray(np.tile(bq.reshape(C, 1), (2, 1)))
    # every partition = concat(bk, bv) -> [128, 128]
    bkv = np.ascontiguousarray(
        np.tile(np.concatenate([bk, bv]).reshape(1, 128), (128, 1))
    )
    return {"Wq": Wq_p, "Wkv": Wkv, "bq": bq_p, "bkv": bkv}


def bench_nc_and_inmaps(reps):
    """For bench.py: nc with `reps` in-NEFF repetitions + per-core inputs."""
    rng = np.random.default_rng(0)
    nc = build_nc(hw_workaround=True, reps=reps)
    params = prep_params(
        rng.standard_normal((C, C), dtype=np.float32) * 0.1,
        rng.standard_normal(C).astype(np.float32) * 0.1,
        rng.standard_normal((C, C), dtype=np.float32) * 0.1,
        rng.standard_normal(C).astype(np.float32) * 0.1,
        rng.standard_normal((C, C), dtype=np.float32) * 0.1,
        rng.standard_normal(C).astype(np.float32) * 0.1,
    )
    x = rng.standard_normal((C, HW), dtype=np.float32)
    in_maps = [
        {"q": x, "k": x, "v": x, **params} for _ in range(B)
    ]
    return nc, in_maps


def kernel(q, k, v, Wq, bq, Wk, bk, Wv, bv):
    q = np.ascontiguousarray(np.asarray(q), dtype=np.float32)
    k = np.ascontiguousarray(np.asarray(k), dtype=np.float32)
    v = np.ascontiguousarray(np.asarray(v), dtype=np.float32)
    params = prep_params(Wq, bq, Wk, bk, Wv, bv)

    nc = _get_nc()
    in_maps = []
    for b in range(B):
        in_maps.append(
            {
                "q": q[b].reshape(C, HW),
                "k": k[b].reshape(C, HW),
                "v": v[b].reshape(C, HW),
                **params,
            }
        )
    res = run_bass_kernel_spmd(nc, in_maps, list(range(B)), **_RUN_KWARGS)
    global LAST_RESULT
    LAST_RESULT = res
    out = np.stack([res.results[b]["out"].reshape(C, H, W) for b in range(B)])
    return out



# revision 3
# speedup vs baseline: 2.6006x; 2.6006x over previous
"""Trainium2 Bass kernel for nn_MultiHeadAttention_7954279432294 (v2, bf16).

Reference computation (per batch b, row h; C=64 channels, W=512 width):
    qp = q^T Wq^T + bq       [W, C]
    kp = k^T Wk^T + bk       [W, C]
    vp = v^T Wv^T + bv       [W, C]
    out = (qp @ kp^T) @ vp   [W, C]   (linear attention, NO softmax)
    result = out^T + q       [C, W]   (NCHW + residual)

Algebra (v2): no softmax =>
    out = qp @ S            where  S = kp^T vp           [C, C]
        = Q^T (Wq^T S) + 1 (bq^T S)
so with  M := Wq^T S  [C,C]  and  r := S^T bq  [C]:
    result = (M + I)^T Q + r 1^T     -- residual folds into M via +I.
The q-projection is never materialized: the only W-sized matmuls are the
k/v projections (which double as the W-onto-partitions transpose needed
by S) and the (M+I)^T Q output matmuls.

dtype: bf16 on the wire and in matmul operands (fp32 PSUM accumulation).
Host casts fp32->bf16: halves HBM traffic, enables FWL weight loads.
Measured pipeline rel-err vs fp64 reference: ~5e-3 (gate is 2e-2).

DMA: k and v are concatenated host-side into one [128, HW] tensor so
each chunk's k+v is a single 2 MB 128-partition DMA on the SP HWDGE
ring; q chunks are single 1 MB DMAs on the ACT HWDGE ring (3-level AP);
outputs go out through the Pool/SWDGE ring. Three rings overlap their
per-DMA completion-receipt latencies (~2 us each).

Pipeline: the per-pair work is software-pipelined one pair deep: the
PE-side tail (S/M/r/out matmuls) of pair i is emitted after the head
(projections) of pair i+1, so the PE never stalls waiting for the DVE
PSUM->SBUF copy of its own pair.

Sharding: data-parallel over B (8 batches over 8 cores), no comms.

Layout per core: H=64 rows as 32 pairs (h, h+8); the pair's channels
fill the 128 SBUF partitions (group 0 = h -> 0:64, group 1 = h+8 ->
64:128). Projection matmuls use the k|v data chunk as the 128x128
stationary (full PE array, FWL) and block-diag weights as the moving
operand, producing pv blocks [w(128 partitions), kp|vp(128)] laid out
interleaved [j0g0 | j0g1 | j1g0 | ...] so one DVE op applies bias+cast
to the whole pair. S matmuls use the kp half-block as a 64-col
stationary; group 1's S lands on PSUM partitions 64:128 via column
tile position, so a single [128, 64] copy stages both S matrices.
"""

import numpy as np
import ml_dtypes

import concourse.bass as bass
import concourse.mybir as mybir
import concourse.tile as tile
from concourse.bass_utils import run_bass_kernel_spmd

B, C, H, W = 8, 64, 64, 512
USES_KV_CONCAT = True
HW = H * W
F32 = mybir.dt.float32
BF16 = mybir.dt.bfloat16

N_CHUNK = 4
H_PER_CHUNK = H // N_CHUNK          # 16
PAIRS_PER_CHUNK = H_PER_CHUNK // 2  # 8
CHUNK_F = PAIRS_PER_CHUNK * W       # 4096

# const blob column layouts
_CB_BF = {"wkv": (0, 128), "wq2": (128, 128), "bq2": (256, 1)}
CB_BF_COLS = 257
_CB_F32 = {"bkv": (0, 128), "i2": (128, 64)}
CB_F32_COLS = 192


def _add_bcast(nc, out_ap, in0_ap, bias_ap, reps, width):
    """out = in0 + bias, bias [128, width] broadcast `reps` times along the
    free dim (out/in0 are [128, reps*width])."""
    out3 = out_ap.rearrange("p (r c) -> p r c", c=width)
    in03 = in0_ap.rearrange("p (r c) -> p r c", c=width)
    bias3 = bass.AP(
        tensor=bias_ap.tensor,
        offset=bias_ap.offset,
        ap=[bias_ap.ap[0], [0, reps], bias_ap.ap[1]],
    )
    nc.vector.tensor_tensor(
        out=out3, in0=in03, in1=bias3, op=mybir.AluOpType.add
    )


def build_nc(hw_workaround: bool = False, reps: int = 1) -> bass.Bass:
    nc = bass.Bass()

    q_d = nc.declare_dram_parameter("q", [C, HW], BF16, isOutput=False)
    kv_d = nc.declare_dram_parameter("kv", [128, HW], BF16, isOutput=False)
    cb_bf_d = nc.declare_dram_parameter("cb_bf", [128, CB_BF_COLS], BF16,
                                        isOutput=False)
    cb_f32_d = nc.declare_dram_parameter("cb_f32", [128, CB_F32_COLS], F32,
                                         isOutput=False)
    out_d = nc.declare_dram_parameter("out", [C, HW], BF16, isOutput=True)

    def q_ap(ch):
        """[C, 8192] DRAM chunk -> [g=2, c=64, f=4096] AP matching a
        [128, 4096] SBUF tile's (partition = g*64+c) iteration order."""
        base = q_d[:, :]
        return bass.AP(
            tensor=base.tensor,
            offset=base.offset + ch * 2 * CHUNK_F,
            ap=[[CHUNK_F, 2], [HW, C], [1, CHUNK_F]],
        )

    with tile.TileContext(nc) as tc:
        with (
            tc.tile_pool(name="const", bufs=1) as const,
            tc.tile_pool(name="io", bufs=3) as io,
            tc.tile_pool(name="mid", bufs=2) as mid,
            tc.tile_pool(name="psA", bufs=2, space="PSUM") as psA,
            tc.tile_pool(name="psB", bufs=2, space="PSUM") as psB,
        ):
            # ---------------- consts: two blob DMAs ----------------
            cb_bf = const.tile([128, CB_BF_COLS], BF16)
            nc.sync.dma_start(out=cb_bf[:, :], in_=cb_bf_d[:, :])
            cb_f32 = const.tile([128, CB_F32_COLS], F32)
            nc.scalar.dma_start(out=cb_f32[:, :], in_=cb_f32_d[:, :])

            def _sl(t, spec, name):
                o, w = spec[name]
                return t[:, o : o + w]

            wkv = _sl(cb_bf, _CB_BF, "wkv")
            wq2 = _sl(cb_bf, _CB_BF, "wq2")
            bq2 = _sl(cb_bf, _CB_BF, "bq2")
            bkv = _sl(cb_f32, _CB_F32, "bkv")
            i2 = _sl(cb_f32, _CB_F32, "i2")

            lo, hi = slice(0, C), slice(C, 128)

            def head(ctx):
                """projections + bias/cast for one pair."""
                kv_sb, hp = ctx["kv_sb"], ctx["hp"]
                pv_ps = psA.tile([128, 1024], F32, tag="pv_ps")
                for g in range(2):
                    for j in range(4):
                        wsl = slice(g * CHUNK_F + hp * W + j * 128,
                                    g * CHUNK_F + hp * W + (j + 1) * 128)
                        nc.tensor.matmul(
                            pv_ps[:, (2 * j + g) * 128 : (2 * j + g + 1) * 128],
                            kv_sb[:, wsl], wkv,
                            start=True, stop=True,
                        )
                pv_sb = mid.tile([128, 1024], BF16, tag="pv_sb")
                _add_bcast(nc, pv_sb[:, :], pv_ps[:, :], bkv, 8, width=128)
                ctx["pv_sb"] = pv_sb

            def tail(ctx):
                """S, M+I, r, out matmuls + stores for one pair."""
                pv_sb, q_sb, o_sb, hp = (ctx["pv_sb"], ctx["q_sb"],
                                         ctx["o_sb"], ctx["hp"])
                hs = slice(hp * W, (hp + 1) * W)
                psB_t = psB.tile([128, 129], F32, tag="psB_t")
                psS = psB_t[:, 0:64]
                m2_ps = psB_t[:, 64:128]
                r2_ps = psB_t[:, 128:129]
                # S_h = kp^T vp accumulated over 4 w-chunks; 64-col kp
                # stationary, vp moving; h2 -> partitions 64:128 (col tile)
                for g, gp in ((0, lo), (1, hi)):
                    for j in range(4):
                        b0 = (2 * j + g) * 128
                        nc.tensor.matmul(
                            psS[gp, :],
                            pv_sb[:, b0 : b0 + 64],
                            pv_sb[:, b0 + 64 : b0 + 128],
                            start=(j == 0), stop=(j == 3),
                        )
                S_sb = mid.tile([128, C], BF16, tag="S_sb")
                nc.scalar.copy(S_sb[:, :], psS[:, :])

                # M = Wq^T S (both h at once, block-diag Wq2)
                nc.tensor.matmul(m2_ps, wq2, S_sb[:, :], start=True, stop=True)
                # r = S^T bq (per h; (64,64) tile for h2)
                nc.tensor.matmul(r2_ps[lo, :], S_sb[lo, :], bq2[lo, :],
                                 start=True, stop=True)
                nc.tensor.matmul(r2_ps[hi, :], S_sb[hi, :], bq2[hi, :],
                                 start=True, stop=True)
                # M' = M + I (DVE, one op, both halves) -> bf16 stationary
                m2v = mid.tile([128, C], BF16, tag="m2v")
                nc.vector.tensor_tensor(out=m2v[:, :], in0=m2_ps, in1=i2,
                                        op=mybir.AluOpType.add)
                r2_sb = mid.tile([128, 1], F32, tag="r2_sb")
                nc.scalar.copy(r2_sb[:, :], r2_ps[:, :])

                # out^T = M'^T Q per h (+ r bias in the ACT copy)
                out_ps = psA.tile([128, 512], F32, tag="out_ps")
                nc.tensor.matmul(out_ps[lo, :], m2v[lo, :], q_sb[lo, hs],
                                 start=True, stop=True)
                nc.tensor.matmul(out_ps[hi, :], m2v[hi, :], q_sb[hi, hs],
                                 start=True, stop=True)
                nc.scalar.add(o_sb[:, hs], out_ps[:, :], add=r2_sb[:, :])
                ctx["done"] = True

            def flush_out_dma(ctx):
                o_sb, ch = ctx["o_sb"], ctx["ch"]
                lo2 = ch * 2 * CHUNK_F
                for g, gp in ((0, lo), (1, hi)):
                    nc.gpsimd.dma_start(
                        out=out_d[:, lo2 + g * CHUNK_F : lo2 + (g + 1) * CHUNK_F],
                        in_=o_sb[gp, :],
                    )

            # ---------------- main loop (1-pair software pipeline) ------
            pending = None
            for ch in [c for _ in range(reps) for c in range(N_CHUNK)]:
                kv_sb = io.tile([128, 2 * CHUNK_F], BF16, tag="kv_sb")
                q_sb = io.tile([128, CHUNK_F], BF16, tag="q_sb")
                o_sb = io.tile([128, CHUNK_F], BF16, tag="o_sb")
                lo2 = ch * 2 * CHUNK_F
                nc.sync.dma_start(out=kv_sb[:, :],
                                  in_=kv_d[:, lo2 : lo2 + 2 * CHUNK_F])
                nc.scalar.dma_start(out=q_sb[:, :], in_=q_ap(ch))

                for hp in range(PAIRS_PER_CHUNK):
                    ctx = {"kv_sb": kv_sb, "q_sb": q_sb, "o_sb": o_sb,
                           "hp": hp, "ch": ch, "last": hp == PAIRS_PER_CHUNK - 1}
                    head(ctx)
                    if pending is not None:
                        tail(pending)
                        if pending["last"]:
                            flush_out_dma(pending)
                    pending = ctx
            tail(pending)
            flush_out_dma(pending)

    if hw_workaround:
        _absorb_matmul_waits(nc)
    nc.finalize()
    return nc


def _absorb_matmul_waits(nc):
    """This walrus build rejects any engine instruction carrying more than one
    sync wait. Split an instruction's n waits into n same-engine NoOps (one
    wait each) inserted right before it: engines execute their stream in FIFO
    order, so the instruction stays correctly gated."""
    ctr = 0
    for bb in nc.m.functions[0].blocks:
        insts = bb.instructions
        i = 0
        while i < len(insts):
            inst = insts[i]
            si = inst.sync_info
            if si is not None and si.on_wait and len(si.on_wait) > 1:
                for w in si.on_wait:
                    nop = mybir.InstNoOp(
                        name=f"I-mmwait-{ctr}", engine=inst.engine, ins=[], outs=[]
                    )
                    ctr += 1
                    nop.sync_info = mybir.SyncInfo(on_wait=[w], on_update=[])
                    insts.insert(i, nop)
                    i += 1
                inst.sync_info = mybir.SyncInfo(
                    on_wait=[], on_update=list(si.on_update)
                )
            i += 1


def to_bf16(x):
    """fp32 -> bf16 with round-to-nearest-even, vectorized (no NaN inputs)."""
    u = np.ascontiguousarray(x, dtype=np.float32).view(np.uint32)
    r = ((u >> 16) & np.uint32(1)) + np.uint32(0x7FFF)
    return ((u + r) >> np.uint32(16)).astype(np.uint16).view(ml_dtypes.bfloat16)


def prep_params(Wq, bq, Wk, bk, Wv, bv):
    Wq = np.asarray(Wq, dtype=np.float32)
    Wk = np.asarray(Wk, dtype=np.float32)
    Wv = np.asarray(Wv, dtype=np.float32)
    bq = np.asarray(bq, dtype=np.float32).reshape(C)
    bk = np.asarray(bk, dtype=np.float32).reshape(C)
    bv = np.asarray(bv, dtype=np.float32).reshape(C)

    z = np.zeros((C, C), dtype=np.float32)
    cb_bf = np.zeros((128, CB_BF_COLS), dtype=ml_dtypes.bfloat16)

    def put_bf(name, arr):
        o, w = _CB_BF[name]
        cb_bf[:, o : o + w] = to_bf16(arr).reshape(128, w)

    put_bf("wkv", np.block([[Wk.T, z], [z, Wv.T]]))
    put_bf("wq2", np.block([[Wq, z], [z, Wq]]))
    put_bf("bq2", np.tile(bq.reshape(C, 1), (2, 1)))

    cb_f32 = np.zeros((128, CB_F32_COLS), dtype=np.float32)

    def put_f32(name, arr):
        o, w = _CB_F32[name]
        cb_f32[:, o : o + w] = arr

    put_f32("bkv", np.tile(np.concatenate([bk, bv]).reshape(1, 128), (128, 1)))
    put_f32("i2", np.vstack([np.eye(C, dtype=np.float32)] * 2))
    return {"cb_bf": cb_bf, "cb_f32": cb_f32}


_NC_CACHE = None
_RUN_KWARGS = {}
LAST_RESULT = None
# bench.py burst+paired differential measurement (see bench.py):
# baseline kernel ~200 us, this kernel ~61.6 us under identical methodology.
MEASURED_EXEC_NS = 61604


def _get_nc():
    global _NC_CACHE
    if _NC_CACHE is None:
        _NC_CACHE = build_nc(hw_workaround=True)
    return _NC_CACHE


def kernel(q, k, v, Wq, bq, Wk, bk, Wv, bv):
    qb = to_bf16(np.asarray(q)).reshape(B, C, HW)
    kb = to_bf16(np.asarray(k)).reshape(B, C, HW)
    vb = to_bf16(np.asarray(v)).reshape(B, C, HW)
    kvb = np.concatenate([kb, vb], axis=1)  # [B, 128, HW]
    params = prep_params(Wq, bq, Wk, bk, Wv, bv)

    nc = _get_nc()
    in_maps = [{"q": qb[b], "kv": kvb[b], **params} for b in range(B)]
    res = run_bass_kernel_spmd(nc, in_maps, list(range(B)), **_RUN_KWARGS)
    global LAST_RESULT
    LAST_RESULT = res
    out = np.stack(
        [res.results[b]["out"].astype(np.float32).reshape(C, H, W)
         for b in range(B)]
    )
    return out


def bench_nc_and_inmaps(reps):
    """For bench.py/probe.py: nc with `reps` in-NEFF repetitions + inputs."""
    rng = np.random.default_rng(0)
    nc = build_nc(hw_workaround=True, reps=reps)
    params = prep_params(
        *(rng.standard_normal(s).astype(np.float32) * 0.1
          for s in ((C, C), C, (C, C), C, (C, C), C))
    )
    x = to_bf16(rng.standard_normal((C, HW), dtype=np.float32))
    kv = to_bf16(rng.standard_normal((128, HW), dtype=np.float32))
    in_maps = [{"q": x, "kv": kv, **params} for _ in range(B)]
    return nc, in_maps
